# revision 9
# baseline (speedup 1.0000x reference)
"""EnsemblePrompt retrieval-knn kernel for 8 Trainium2 NeuronCores.

Pipeline per core (data-parallel over batch, 128 rows/core):
  score   = q @ proj                       [128, 4096]  fp32 PE matmul
  t*      = 204th-largest per row          bisection with exact fp32 counting on DVE
  sparse  = score * (score >= t*)          one fused DVE pass
  raw     = sparse @ key_bank.T            fp32 PE matmul (lhsT = PE-transposed sparse)
  v       = raw * (1/||kb_p||)             key-bank norms computed on device (ACT)
  top8    = max8/max_index on v            DVE top-k instructions -> sel_idx + values
  simil   = 1 - v_sel / ||sparse||
  gather  = prompts[sel_idx] via gpsimd dma_gather, fanned out to the 4 identical pools

The 4 prompt pools are identical by construction (jnp.tile in setup_inputs); the
kernel gathers from pool 0 once and writes all 4 output pool copies.  A host-side
equality check falls back to per-pool device runs if they ever differ.
"""

import numpy as np

N_CORES = 8
B, D, E, P = 1024, 768, 4096, 1024
L, S, POOLS = 8, 8, 4
NUM_ACTIVE = 204
BC = B // N_CORES          # 128 batch rows per core
LD = L * D                 # 6144 floats per prompt row
EPS = 1e-8
BISECT_ITERS = 27          # numpy mirror converges in 22 on the reference seed
GATHER_CHUNKS = 4          # 256 gathered rows per dma_gather call

_CACHE = {}


def _build_nc():
    import concourse.mybir as mybir
    from concourse import bacc
    from concourse.tile import TileContext

    f32 = mybir.dt.float32
    nc = bacc.Bacc("TRN2", target_bir_lowering=False)

    qT = nc.declare_dram_parameter("qT", [D, BC], f32, isOutput=False)
    proj = nc.declare_dram_parameter("proj", [D, E], f32, isOutput=False)
    kb = nc.declare_dram_parameter("kb", [P, E], f32, isOutput=False)
    kbT = nc.declare_dram_parameter("kbT", [E, P], f32, isOutput=False)
    pr0 = nc.declare_dram_parameter("pr0", [P, LD], f32, isOutput=False)
    sim_out = nc.declare_dram_parameter("sim_out", [BC, S], f32, isOutput=True)
    sel_out = nc.declare_dram_parameter("sel_out", [POOLS, BC * S, LD], f32, isOutput=True)
    idx_dram = nc.dram_tensor("idx_dram", [BC * S], mybir.dt.int16)

    with TileContext(nc) as tc:
        _emit(tc, nc, mybir, qT, proj, kb, kbT, pr0, sim_out, sel_out, idx_dram)
    if not nc.is_finalized():
        nc.finalize()
    return nc


def _emit(tc, nc, mybir, qT, proj, kb, kbT, pr0, sim_out, sel_out, idx_dram):
    from contextlib import ExitStack
    from concourse.bass import ts
    from concourse.masks import make_identity

    f32 = mybir.dt.float32
    Alu = mybir.AluOpType
    Act = mybir.ActivationFunctionType
    KC = D // 128            # 6 contraction chunks for score
    NSC = E // 512           # 8 n-chunks of score
    EC = E // 128            # 32 chunks of E
    PC = P // 128            # 8 chunks of P

    with ExitStack() as ctx:
        const_pool = ctx.enter_context(tc.tile_pool(name="const", bufs=1))
        ident = const_pool.tile([128, 128], f32)
        make_identity(nc, ident)

        persist = ctx.enter_context(tc.tile_pool(name="persist", bufs=1))
        score = persist.tile([128, E], f32)
        sparse = persist.tile([128, E], f32)
        sparseT = persist.tile([128, EC, 128], f32)
        inv_p = persist.tile([128, P], f32)
        v = persist.tile([128, P], f32)
        cnt_dump = persist.tile([128, E], f32)    # indicator dump for bisection
        act_dump = persist.tile([128, E], f32)    # ACT output dump for kb norms

        # ---------------- load qT (stationary operands) ----------------
        qpool = ctx.enter_context(tc.tile_pool(name="qtp", bufs=1))
        qts = []
        for k in range(KC):
            qt_k = qpool.tile([128, BC], f32, name=f"qt{k}", tag=f"qt{k}")
            nc.sync.dma_start(qt_k, qT[ts(k, 128), :])
            qts.append(qt_k)

        # ---------------- score = qT.T @ proj ----------------
        with tc.tile_pool(name="projp", bufs=3) as proj_pool, \
             tc.tile_pool(name="pscore", bufs=1, space="PSUM") as ps_pool:
            ps = [ps_pool.tile([128, 512], f32, name=f"ps{n}", tag=f"ps{n}")
                  for n in range(NSC)]
            for k in range(KC):
                pj = proj_pool.tile([128, E], f32, name="pj", tag="pj")
                nc.sync.dma_start(pj, proj[ts(k, 128), :])
                for n in range(NSC):
                    nc.tensor.matmul(ps[n], qts[k], pj[:, ts(n, 512)],
                                     start=(k == 0), stop=(k == KC - 1))
            for n in range(NSC):
                nc.vector.tensor_copy(score[:, ts(n, 512)], ps[n])

        # ---------------- key-bank norms (ACT, overlaps score/bisection) ----------------
        nsq_kb = persist.tile([128, PC], f32)
        with tc.tile_pool(name="kbrows", bufs=2) as kbrow_pool:
            for i in range(PC):
                kbr = kbrow_pool.tile([128, E], f32, name="kbr", tag="kbr")
                nc.sync.dma_start(kbr, kb[ts(i, 128), :])
                nc.scalar.activation(act_dump, kbr, Act.Square,
                                     accum_out=nsq_kb[:, i:i + 1])
        kn_norm = persist.tile([128, PC], f32)
        nc.scalar.activation(kn_norm, nsq_kb, Act.Sqrt)
        nc.vector.tensor_scalar_max(kn_norm, kn_norm, EPS)
        inv_kn = persist.tile([128, PC], f32)
        nc.vector.reciprocal(inv_kn, kn_norm)

        # [128, 8] -> [8, 128] -> [1, 1024] -> broadcast to [128, 1024]
        # via a K=1 PE outer product (ones[1,128] as lhsT).
        with tc.tile_pool(name="pinv", bufs=1, space="PSUM") as pinv_pool:
            inv_ps = pinv_pool.tile([PC, 128], f32)
            nc.tensor.transpose(inv_ps, inv_kn, ident)
            inv_knT = persist.tile([PC, 128], f32)
            nc.vector.tensor_copy(inv_knT, inv_ps)
            inv_row = persist.tile([1, P], f32)
            nc.sync.dma_start(inv_row.rearrange("a (c i) -> a c i", c=PC), inv_knT)
            ones_row = persist.tile([1, 128], f32)
            nc.vector.memset(ones_row, 1.0)
            for n in range(2):
                inv_bc = pinv_pool.tile([128, 512], f32, name="inv_bc", tag="inv_bc")
                nc.tensor.matmul(inv_bc, ones_row, inv_row[:, ts(n, 512)],
                                 start=True, stop=True)
                nc.vector.tensor_copy(inv_p[:, ts(n, 512)], inv_bc)

        # ---------------- bisection for the 204th largest ----------------
        bis = ctx.enter_context(tc.tile_pool(name="bis", bufs=2))
        rs = bis.tile([128, 1], f32, name="rs", tag="rs", bufs=1)
        nc.vector.reduce_sum(rs, score, axis=mybir.AxisListType.X)
        lo = bis.tile([128, 1], f32, name="lo", tag="lo")
        nc.vector.tensor_scalar_mul(lo, rs, 1.0 / E)        # lo = rowmean
        hi = bis.tile([128, 1], f32, name="hi", tag="hi")
        nc.vector.reduce_max(hi, score, axis=mybir.AxisListType.X)

        for _ in range(BISECT_ITERS):
            mid = bis.tile([128, 1], f32, name="mid", tag="mid")
            nc.vector.tensor_add(mid, lo, hi)
            nc.vector.tensor_scalar_mul(mid, mid, 0.5)
            cnt = bis.tile([128, 1], f32, name="cnt", tag="cnt")
            nc.vector.tensor_scalar(cnt_dump, score, mid, None,
                                    op0=Alu.is_ge, op1=Alu.add, accum_out=cnt)
            g = bis.tile([128, 1], mybir.dt.uint8, name="g", tag="g")
            nc.vector.tensor_scalar(g, cnt, float(NUM_ACTIVE) - 0.5, None,
                                    op0=Alu.is_ge)
            lo2 = bis.tile([128, 1], f32, name="lo2", tag="lo")
            nc.vector.select(lo2, g, mid, lo)               # count>=204: lo=mid
            hi2 = bis.tile([128, 1], f32, name="hi2", tag="hi")
            nc.vector.select(hi2, g, hi, mid)               # else: hi=mid
            lo, hi = lo2, hi2

        tpos = lo

        # ---------------- sparse + its norm ----------------
        nc.vector.scalar_tensor_tensor(sparse, score, tpos, score,
                                       op0=Alu.is_ge, op1=Alu.mult)
        nsq_sp = bis.tile([128, 1], f32, name="nsq_sp", tag="nsq_sp", bufs=1)
        nc.vector.tensor_mul(cnt_dump, sparse, sparse)
        nc.vector.reduce_sum(nsq_sp, cnt_dump, axis=mybir.AxisListType.X)
        sp_norm = bis.tile([128, 1], f32, name="sp_norm", tag="sp_norm", bufs=1)
        nc.scalar.activation(sp_norm, nsq_sp, Act.Sqrt)
        nc.vector.tensor_scalar_max(sp_norm, sp_norm, EPS)
        inv_sp = bis.tile([128, 1], f32, name="inv_sp", tag="inv_sp", bufs=1)
        nc.vector.reciprocal(inv_sp, sp_norm)

        # ---------------- transpose sparse (PE) ----------------
        with tc.tile_pool(name="ptr", bufs=2, space="PSUM") as ptr_pool:
            for grp in range(EC // 4):
                pst4 = ptr_pool.tile([128, 512], f32, name="pst4", tag="pst4")
                for j in range(4):
                    e = grp * 4 + j
                    nc.tensor.transpose(pst4[:, ts(j, 128)],
                                        sparse[:, ts(e, 128)], ident)
                nc.vector.tensor_copy(
                    sparseT[:, grp * 4:(grp + 1) * 4, :].rearrange("p c i -> p (c i)"),
                    pst4)

        # ---------------- raw = sparseT.T @ kbT ; v = raw * inv_p ----------------
        with tc.tile_pool(name="kbtp", bufs=6) as kbt_pool, \
             tc.tile_pool(name="pm", bufs=1, space="PSUM") as pm_pool:
            psm = [pm_pool.tile([128, 512], f32, name=f"psm{n}", tag=f"psm{n}")
                   for n in range(2)]
            for k in range(EC):
                kt = kbt_pool.tile([128, P], f32, name="kt", tag="kt")
                nc.sync.dma_start(kt, kbT[ts(k, 128), :])
                for n in range(2):
                    nc.tensor.matmul(psm[n], sparseT[:, k, :], kt[:, ts(n, 512)],
                                     start=(k == 0), stop=(k == EC - 1))
            for n in range(2):
                nc.vector.tensor_tensor(v[:, ts(n, 512)], psm[n],
                                        inv_p[:, ts(n, 512)], op=Alu.mult)

        # ---------------- top-8 + similarity ----------------
        max8 = persist.tile([128, S], f32)
        idx8 = persist.tile([128, S], mybir.dt.uint32)
        nc.vector.max(max8, v)
        nc.vector.max_index(idx8, max8, v)
        simt = persist.tile([128, S], f32)
        nc.vector.tensor_scalar(simt, max8, inv_sp, None, op0=Alu.mult)
        nc.vector.tensor_scalar(simt, simt, -1.0, 1.0, op0=Alu.mult, op1=Alu.add)
        nc.sync.dma_start(sim_out[:], simt)

        # ---------------- index plumbing for the gather ----------------
        idx16 = persist.tile([128, S], mybir.dt.int16)
        nc.vector.tensor_copy(idx16, idx8)
        nc.sync.dma_start(idx_dram[:].rearrange("(b s) -> b s", s=S), idx16)
        idxw = persist.tile([128, (BC * S) // 16], mybir.dt.int16)
        flat16 = idx_dram[:].rearrange("(s p) -> p s", p=16)  # [16, 64] wrapped view
        for grp in range(8):
            nc.sync.dma_start(idxw[ts(grp, 16), :], flat16)

        # ---------------- gather + 4-pool fan-out ----------------
        rows_per = (BC * S) // GATHER_CHUNKS                # 256
        slots = rows_per // 128                             # 2
        gpool = ctx.enter_context(tc.tile_pool(name="gath", bufs=2))
        for c in range(GATHER_CHUNKS):
            gt = gpool.tile([128, slots, LD], f32, name="gt", tag="gt")
            nc.gpsimd.dma_gather(
                out_ap=gt, in_ap=pr0[:],
                idxs_ap=idxw[:, c * (rows_per // 16):(c + 1) * (rows_per // 16)],
                num_idxs=rows_per, num_idxs_reg=rows_per, elem_size=LD)
            for pi in range(POOLS):
                dest = sel_out[pi, c * rows_per:(c + 1) * rows_per, :] \
                    .rearrange("(c2 p) e -> p c2 e", p=128)
                nc.sync.dma_start(dest, gt)


def _get_nc():
    if "nc" not in _CACHE:
        _CACHE["nc"] = _build_nc()
    return _CACHE["nc"]


def _run_device(q, proj, kb, pr0_flat):
    """One SPMD run over 8 cores. Returns (similarity [B,S], sel_rows [POOLS,B*S,LD] per-core list)."""
    from concourse.bass_utils import run_bass_kernel_spmd

    nc = _get_nc()
    kbT = np.ascontiguousarray(kb.T)
    in_maps = []
    for c in range(N_CORES):
        qs = q[c * BC:(c + 1) * BC]
        in_maps.append({
            "qT": np.ascontiguousarray(qs.T),
            "proj": proj,
            "kb": kb,
            "kbT": kbT,
            "pr0": pr0_flat,
        })
    res = run_bass_kernel_spmd(nc, in_maps, core_ids=list(range(N_CORES)))
    sims = np.concatenate([r["sim_out"] for r in res.results], axis=0)
    sel = np.concatenate(
        [r["sel_out"].reshape(POOLS, BC, S * L, D) for r in res.results], axis=1)
    return sims, sel


def kernel(query, random_projection, key_bank, prompts):
    q = np.ascontiguousarray(query, dtype=np.float32)
    proj = np.ascontiguousarray(random_projection, dtype=np.float32)
    kb = np.ascontiguousarray(key_bank, dtype=np.float32)
    prompts = np.asarray(prompts, dtype=np.float32)

    pools_same = all(np.array_equal(prompts[0], prompts[i]) for i in range(1, POOLS))
    pr0 = np.ascontiguousarray(prompts[0].reshape(P, LD))
    sims, sel = _run_device(q, proj, kb, pr0)
    if not pools_same:
        # Fallback (never hit with the reference setup_inputs): rerun per pool
        # and keep only that pool's slice from each run.
        for pi in range(1, POOLS):
            pri = np.ascontiguousarray(prompts[pi].reshape(P, LD))
            _, sel_i = _run_device(q, proj, kb, pri)
            sel[pi] = sel_i[pi]
    return sims, sel


# revision 14
# speedup vs baseline: 1.1670x; 1.1670x over previous
"""EnsemblePrompt retrieval-knn kernel for 8 Trainium2 NeuronCores.

Pipeline per core (data-parallel over batch, 128 rows/core):
  score   = q @ proj                       [128, 4096]  fp32 PE matmul
  t*      = 204th-largest per row          bisection with exact fp32 counting on DVE
  sparse  = score * (score >= t*)          one fused DVE pass
  raw     = sparse @ key_bank.T            fp32 PE matmul (lhsT = PE-transposed sparse)
  v       = raw * (1/||kb_p||)             key-bank norms computed on device (ACT)
  top8    = max8/max_index on v            DVE top-k instructions -> sel_idx + values
  simil   = 1 - v_sel / ||sparse||
  gather  = prompts[sel_idx] via gpsimd dma_gather, fanned out to the 4 identical pools

The 4 prompt pools are identical by construction (jnp.tile in setup_inputs); the
kernel gathers from pool 0 once and writes all 4 output pool copies.  A host-side
equality check falls back to per-pool device runs if they ever differ.
"""

import numpy as np

N_CORES = 8
B, D, E, P = 1024, 768, 4096, 1024
L, S, POOLS = 8, 8, 4
NUM_ACTIVE = 204
BC = B // N_CORES          # 128 batch rows per core
LD = L * D                 # 6144 floats per prompt row
EPS = 1e-8
BISECT_ITERS = 24          # numpy mirror converges in 22 on the reference seed
ACT_SPLIT = 2176           # bisection count: ACT handles [0:2176], DVE [2176:4096]
GATHER_CHUNKS = 4          # 256 gathered rows per dma_gather call

_CACHE = {}


def _build_nc():
    import concourse.mybir as mybir
    from concourse import bacc
    from concourse.tile import TileContext

    f32 = mybir.dt.float32
    nc = bacc.Bacc("TRN2", target_bir_lowering=False)

    qT = nc.declare_dram_parameter("qT", [D, BC], f32, isOutput=False)
    proj = nc.declare_dram_parameter("proj", [D, E], f32, isOutput=False)
    kb = nc.declare_dram_parameter("kb", [P, E], f32, isOutput=False)
    kbT = nc.declare_dram_parameter("kbT", [E, P], f32, isOutput=False)
    pr0 = nc.declare_dram_parameter("pr0", [P, LD], f32, isOutput=False)
    sim_out = nc.declare_dram_parameter("sim_out", [BC, S], f32, isOutput=True)
    sel_out = nc.declare_dram_parameter("sel_out", [POOLS, BC * S, LD], f32, isOutput=True)
    idx_dram = nc.dram_tensor("idx_dram", [BC * S], mybir.dt.int16)

    with TileContext(nc) as tc:
        _emit(tc, nc, mybir, qT, proj, kb, kbT, pr0, sim_out, sel_out, idx_dram)
    if not nc.is_finalized():
        nc.finalize()
    return nc


def _emit(tc, nc, mybir, qT, proj, kb, kbT, pr0, sim_out, sel_out, idx_dram):
    from contextlib import ExitStack
    from concourse.bass import ts
    from concourse.masks import make_identity

    f32 = mybir.dt.float32
    Alu = mybir.AluOpType
    Act = mybir.ActivationFunctionType
    KC = D // 128            # 6 contraction chunks for score
    NSC = E // 512           # 8 n-chunks of score
    EC = E // 128            # 32 chunks of E
    PC = P // 128            # 8 chunks of P

    with ExitStack() as ctx:
        const_pool = ctx.enter_context(tc.tile_pool(name="const", bufs=1))
        ident = const_pool.tile([128, 128], f32)
        make_identity(nc, ident)

        persist = ctx.enter_context(tc.tile_pool(name="persist", bufs=1))
        score = persist.tile([128, E], f32)
        sparse = persist.tile([128, E], f32)
        sparseT = persist.tile([128, EC, 128], f32)
        inv_p = persist.tile([128, P], f32)
        v = persist.tile([128, P], f32)
        dumpA = persist.tile([128, ACT_SPLIT], f32)   # ACT bisection dump
        dumpB = persist.tile([128, E], f32)           # DVE dump (norms/bisect/normsq)

        # ---------------- load qT (stationary operands) ----------------
        qpool = ctx.enter_context(tc.tile_pool(name="qtp", bufs=1))
        qts = []
        for k in range(KC):
            qt_k = qpool.tile([128, BC], f32, name=f"qt{k}", tag=f"qt{k}")
            nc.sync.dma_start(qt_k, qT[ts(k, 128), :])
            qts.append(qt_k)

        # PE warm-up: ~5us of back-to-back tiny matmuls so the HAM un-gates
        # the clock before the score matmuls start.  (Dead writes survive —
        # the BIR verifier only warns on no-reader locations.)
        with tc.tile_pool(name="pwarm", bufs=1, space="PSUM") as pw_pool:
            psw = pw_pool.tile([128, 128], f32)
            for i in range(48):
                nc.tensor.matmul(psw, qts[0], qts[1], start=True, stop=True)

        # ---------------- key-bank norms on DVE (runs during score phase) ----
        nsq_kb = persist.tile([128, PC], f32)
        with tc.tile_pool(name="kbrows", bufs=2) as kbrow_pool:
            for i in range(PC):
                kbr = kbrow_pool.tile([128, E], f32, name="kbr", tag="kbr")
                nc.sync.dma_start(kbr, kb[ts(i, 128), :])
                nc.vector.scalar_tensor_tensor(dumpB, kbr, 1.0, kbr,
                                               op0=Alu.bypass, op1=Alu.mult,
                                               accum_out=nsq_kb[:, i:i + 1])

        # ---------------- score = qT.T @ proj ----------------
        with tc.tile_pool(name="projp", bufs=3) as proj_pool, \
             tc.tile_pool(name="pscore", bufs=1, space="PSUM") as ps_pool:
            ps = [ps_pool.tile([128, 512], f32, name=f"ps{n}", tag=f"ps{n}")
                  for n in range(NSC)]
            for k in range(KC):
                pj = proj_pool.tile([128, E], f32, name="pj", tag="pj")
                nc.sync.dma_start(pj, proj[ts(k, 128), :])
                for n in range(NSC):
                    nc.tensor.matmul(ps[n], qts[k], pj[:, ts(n, 512)],
                                     start=(k == 0), stop=(k == KC - 1))
            for n in range(NSC):
                nc.vector.tensor_copy(score[:, ts(n, 512)], ps[n])

        # inv_kn = 1/max(||kb_p||, eps), broadcast to [128, P] via a K=1
        # PE outer product (ones[1,128] as lhsT).
        kn_norm = persist.tile([128, PC], f32)
        nc.scalar.activation(kn_norm, nsq_kb, Act.Sqrt)
        nc.vector.tensor_scalar_max(kn_norm, kn_norm, EPS)
        inv_kn = persist.tile([128, PC], f32)
        nc.vector.reciprocal(inv_kn, kn_norm)
        with tc.tile_pool(name="pinv", bufs=1, space="PSUM") as pinv_pool:
            inv_ps = pinv_pool.tile([PC, 128], f32)
            nc.tensor.transpose(inv_ps, inv_kn, ident)
            inv_knT = persist.tile([PC, 128], f32)
            nc.vector.tensor_copy(inv_knT, inv_ps)
            inv_row = persist.tile([1, P], f32)
            nc.sync.dma_start(inv_row.rearrange("a (c i) -> a c i", c=PC), inv_knT)
            ones_row = persist.tile([1, 128], f32)
            nc.vector.memset(ones_row, 1.0)
            for n in range(2):
                inv_bc = pinv_pool.tile([128, 512], f32, name="inv_bc", tag="inv_bc")
                nc.tensor.matmul(inv_bc, ones_row, inv_row[:, ts(n, 512)],
                                 start=True, stop=True)
                nc.vector.tensor_copy(inv_p[:, ts(n, 512)], inv_bc)

        # ---------------- bisection for the 204th largest ----------------
        # Exact fp32 counting, split ACT (Sign with fused fp32 affine) + DVE.
        bis = ctx.enter_context(tc.tile_pool(name="bis", bufs=2))
        rs = bis.tile([128, 1], f32, name="rs", tag="rs", bufs=1)
        nc.vector.reduce_sum(rs, score, axis=mybir.AxisListType.X)
        lo = bis.tile([128, 1], f32, name="lo", tag="lo", bufs=1)
        nc.vector.tensor_scalar_mul(lo, rs, 1.0 / E)        # lo = rowmean
        hi = bis.tile([128, 1], f32, name="hi", tag="hi", bufs=1)
        nc.vector.reduce_max(hi, score, axis=mybir.AxisListType.X)

        DSPL = E - ACT_SPLIT
        # condition count>=204  <=>  cntD - 0.5*cntA >= 203.5 - ACT_SPLIT/2
        # where cntA = sum(sign(mid - x)) over the ACT half.
        thr = (float(NUM_ACTIVE) - 0.5) - ACT_SPLIT / 2.0
        with tc.tile_pool(name="pkeep", bufs=1, space="PSUM") as pk_pool:
            psk = pk_pool.tile([1, 128], f32)
            for _ in range(BISECT_ITERS):
                mid = bis.tile([128, 1], f32, name="mid", tag="mid")
                nc.vector.tensor_add(mid, lo, hi)
                nc.vector.tensor_scalar_mul(mid, mid, 0.5)
                cntA = bis.tile([128, 1], f32, name="cntA", tag="cntA")
                nc.scalar.activation(dumpA, score[:, :ACT_SPLIT], Act.Sign,
                                     bias=mid, scale=-1.0, accum_out=cntA)
                cntD = bis.tile([128, 1], f32, name="cntD", tag="cntD")
                nc.vector.tensor_scalar(dumpB[:, :DSPL], score[:, ACT_SPLIT:],
                                        mid, None, op0=Alu.is_ge, op1=Alu.add,
                                        accum_out=cntD)
                s_t = bis.tile([128, 1], f32, name="s_t", tag="s_t")
                nc.vector.scalar_tensor_tensor(s_t, cntA, -0.5, cntD,
                                               op0=Alu.mult, op1=Alu.add)
                g = bis.tile([128, 1], mybir.dt.uint8, name="g", tag="g")
                nc.vector.tensor_scalar(g, s_t, thr, None, op0=Alu.is_ge)
                gi = bis.tile([128, 1], mybir.dt.uint8, name="gi", tag="gi")
                nc.vector.tensor_scalar(gi, s_t, thr, None, op0=Alu.is_lt)
                nc.vector.copy_predicated(lo, g, mid)
                nc.vector.copy_predicated(hi, gi, mid)
                # keep the PE HAM warm through the bisection (reads mid so it
                # paces with the loop; result intentionally unused)
                nc.tensor.matmul(psk, mid, score[:, :128], start=True, stop=True)

        tpos = lo

        # ---------------- sparse (chunked so transposes can start early) ----
        for n in range(NSC):
            nc.vector.scalar_tensor_tensor(sparse[:, ts(n, 512)],
                                           score[:, ts(n, 512)], tpos,
                                           score[:, ts(n, 512)],
                                           op0=Alu.is_ge, op1=Alu.mult)

        # -------- interleaved: PE transpose of sparse + match matmul --------
        with tc.tile_pool(name="kbtp", bufs=6) as kbt_pool, \
             tc.tile_pool(name="ptr", bufs=3, space="PSUM") as ptr_pool, \
             tc.tile_pool(name="pm", bufs=1, space="PSUM") as pm_pool:
            psm = [pm_pool.tile([128, 512], f32, name=f"psm{n}", tag=f"psm{n}")
                   for n in range(2)]
            kts = {}
            for grp in range(EC // 4):
                pst4 = ptr_pool.tile([128, 512], f32, name="pst4", tag="pst4")
                for j in range(4):
                    e = grp * 4 + j
                    nc.tensor.transpose(pst4[:, ts(j, 128)],
                                        sparse[:, ts(e, 128)], ident)
                nc.vector.tensor_copy(
                    sparseT[:, grp * 4:(grp + 1) * 4, :].rearrange("p c i -> p (c i)"),
                    pst4)
                for j in range(4):
                    k = grp * 4 + j
                    kt = kbt_pool.tile([128, P], f32, name="kt", tag="kt")
                    nc.sync.dma_start(kt, kbT[ts(k, 128), :])
                    for n in range(2):
                        nc.tensor.matmul(psm[n], sparseT[:, k, :],
                                         kt[:, ts(n, 512)],
                                         start=(k == 0), stop=(k == EC - 1))
            for n in range(2):
                nc.vector.tensor_tensor(v[:, ts(n, 512)], psm[n],
                                        inv_p[:, ts(n, 512)], op=Alu.mult)

        # ---------------- top-8 and gather indices ----------------
        max8 = persist.tile([128, S], f32)
        idx8 = persist.tile([128, S], mybir.dt.uint32)
        nc.vector.max(max8, v)
        nc.vector.max_index(idx8, max8, v)
        idx16 = persist.tile([128, S], mybir.dt.int16)
        nc.vector.tensor_copy(idx16, idx8)
        nc.sync.dma_start(idx_dram[:].rearrange("(b s) -> b s", s=S), idx16)
        idxw = persist.tile([128, (BC * S) // 16], mybir.dt.int16)
        # wrapped [16, 64] view, replicated to all 8 16-partition groups;
        # alternate the two HWDGE rings so the small loads run in parallel
        flat16 = idx_dram[:].rearrange("(s p) -> p s", p=16)
        for grp in range(8):
            eng = nc.sync if grp % 2 == 0 else nc.scalar
            eng.dma_start(idxw[ts(grp, 16), :], flat16)

        # ---------------- gather + 4-pool fan-out ----------------
        rows_per = (BC * S) // GATHER_CHUNKS                # 256
        slots = rows_per // 128                             # 2
        gpool = ctx.enter_context(tc.tile_pool(name="gath", bufs=2))
        for c in range(GATHER_CHUNKS):
            gt = gpool.tile([128, slots, LD], f32, name="gt", tag="gt")
            nc.gpsimd.dma_gather(
                out_ap=gt, in_ap=pr0[:],
                idxs_ap=idxw[:, c * (rows_per // 16):(c + 1) * (rows_per // 16)],
                num_idxs=rows_per, num_idxs_reg=rows_per, elem_size=LD)
            for pi in range(POOLS):
                dest = sel_out[pi, c * rows_per:(c + 1) * rows_per, :] \
                    .rearrange("(c2 p) e -> p c2 e", p=128)
                nc.sync.dma_start(dest, gt)

        # ---------------- similarity output (off the critical path) --------
        nsq_sp = bis.tile([128, 1], f32, name="nsq_sp", tag="nsq_sp", bufs=1)
        nc.vector.scalar_tensor_tensor(dumpB, sparse, 1.0, sparse,
                                       op0=Alu.bypass, op1=Alu.mult,
                                       accum_out=nsq_sp)
        sp_norm = bis.tile([128, 1], f32, name="sp_norm", tag="sp_norm", bufs=1)
        nc.scalar.activation(sp_norm, nsq_sp, Act.Sqrt)
        nc.vector.tensor_scalar_max(sp_norm, sp_norm, EPS)
        inv_sp = bis.tile([128, 1], f32, name="inv_sp", tag="inv_sp", bufs=1)
        nc.vector.reciprocal(inv_sp, sp_norm)
        simt = persist.tile([128, S], f32)
        nc.vector.tensor_scalar(simt, max8, inv_sp, None, op0=Alu.mult)
        nc.vector.tensor_scalar(simt, simt, -1.0, 1.0, op0=Alu.mult, op1=Alu.add)
        nc.sync.dma_start(sim_out[:], simt)


def _get_nc():
    if "nc" not in _CACHE:
        _CACHE["nc"] = _build_nc()
    return _CACHE["nc"]


def _run_device(q, proj, kb, pr0_flat):
    """One SPMD run over 8 cores. Returns (similarity [B,S], sel_rows [POOLS,B*S,LD] per-core list)."""
    from concourse.bass_utils import run_bass_kernel_spmd

    nc = _get_nc()
    kbT = np.ascontiguousarray(kb.T)
    in_maps = []
    for c in range(N_CORES):
        qs = q[c * BC:(c + 1) * BC]
        in_maps.append({
            "qT": np.ascontiguousarray(qs.T),
            "proj": proj,
            "kb": kb,
            "kbT": kbT,
            "pr0": pr0_flat,
        })
    res = run_bass_kernel_spmd(nc, in_maps, core_ids=list(range(N_CORES)))
    sims = np.concatenate([r["sim_out"] for r in res.results], axis=0)
    sel = np.concatenate(
        [r["sel_out"].reshape(POOLS, BC, S * L, D) for r in res.results], axis=1)
    return sims, sel


def kernel(query, random_projection, key_bank, prompts):
    q = np.ascontiguousarray(query, dtype=np.float32)
    proj = np.ascontiguousarray(random_projection, dtype=np.float32)
    kb = np.ascontiguousarray(key_bank, dtype=np.float32)
    prompts = np.asarray(prompts, dtype=np.float32)

    pools_same = all(np.array_equal(prompts[0], prompts[i]) for i in range(1, POOLS))
    pr0 = np.ascontiguousarray(prompts[0].reshape(P, LD))
    sims, sel = _run_device(q, proj, kb, pr0)
    if not pools_same:
        # Fallback (never hit with the reference setup_inputs): rerun per pool
        # and keep only that pool's slice from each run.
        for pi in range(1, POOLS):
            pri = np.ascontiguousarray(prompts[pi].reshape(P, LD))
            _, sel_i = _run_device(q, proj, kb, pri)
            sel[pi] = sel_i[pi]
    return sims, sel


# revision 20
# speedup vs baseline: 1.2441x; 1.0661x over previous
"""EnsemblePrompt retrieval-knn kernel for 8 Trainium2 NeuronCores.

Pipeline per core (data-parallel over batch, 128 rows/core):
  score   = q @ proj                       [128, 4096]  fp32 PE matmul
  t*      = 204th-largest per row          bisection with exact fp32 counting on DVE
  sparse  = score * (score >= t*)          one fused DVE pass
  raw     = sparse @ key_bank.T            fp32 PE matmul (lhsT = PE-transposed sparse)
  v       = raw * (1/||kb_p||)             key-bank norms computed on device (ACT)
  top8    = max8/max_index on v            DVE top-k instructions -> sel_idx + values
  simil   = 1 - v_sel / ||sparse||
  gather  = prompts[sel_idx] via gpsimd dma_gather, fanned out to the 4 identical pools

The 4 prompt pools are identical by construction (jnp.tile in setup_inputs); the
kernel gathers from pool 0 once and writes all 4 output pool copies.  A host-side
equality check falls back to per-pool device runs if they ever differ.
"""

import numpy as np

N_CORES = 8
B, D, E, P = 1024, 768, 4096, 1024
L, S, POOLS = 8, 8, 4
NUM_ACTIVE = 204
BC = B // N_CORES          # 128 batch rows per core
LD = L * D                 # 6144 floats per prompt row
EPS = 1e-8
BISECT_ITERS = 24          # numpy mirror converges in 22 on the reference seed
ACT_SPLIT = 2176           # bisection count: ACT handles [0:2176], DVE [2176:4096]
GATHER_CHUNKS = 8          # 128 gathered rows per dma_gather call

_CACHE = {}


def _build_nc():
    import concourse.mybir as mybir
    from concourse import bacc
    from concourse.tile import TileContext

    f32 = mybir.dt.float32
    nc = bacc.Bacc("TRN2", target_bir_lowering=False)

    qT = nc.declare_dram_parameter("qT", [D, BC], f32, isOutput=False)
    proj = nc.declare_dram_parameter("proj", [D, E], f32, isOutput=False)
    kb = nc.declare_dram_parameter("kb", [P, E], f32, isOutput=False)
    kbT = nc.declare_dram_parameter("kbT", [E, P], f32, isOutput=False)
    pr0 = nc.declare_dram_parameter("pr0", [P, LD], f32, isOutput=False)
    sim_out = nc.declare_dram_parameter("sim_out", [BC, S], f32, isOutput=True)
    sel_out = nc.declare_dram_parameter("sel_out", [POOLS, BC * S, LD], f32, isOutput=True)
    idx_dram = nc.dram_tensor("idx_dram", [BC * S], mybir.dt.int16)

    with TileContext(nc) as tc:
        _emit(tc, nc, mybir, qT, proj, kb, kbT, pr0, sim_out, sel_out, idx_dram)
    if not nc.is_finalized():
        nc.finalize()
    return nc


def _emit(tc, nc, mybir, qT, proj, kb, kbT, pr0, sim_out, sel_out, idx_dram):
    from contextlib import ExitStack
    from concourse.bass import ts
    from concourse.masks import make_identity

    f32 = mybir.dt.float32
    Alu = mybir.AluOpType
    Act = mybir.ActivationFunctionType
    KC = D // 128            # 6 contraction chunks for score
    NSC = E // 512           # 8 n-chunks of score
    EC = E // 128            # 32 chunks of E
    PC = P // 128            # 8 chunks of P

    with ExitStack() as ctx:
        const_pool = ctx.enter_context(tc.tile_pool(name="const", bufs=1))
        ident = const_pool.tile([128, 128], f32)
        make_identity(nc, ident)

        persist = ctx.enter_context(tc.tile_pool(name="persist", bufs=1))
        score = persist.tile([128, E], f32)
        sparse = persist.tile([128, E], f32)
        sparseT = persist.tile([128, EC, 128], f32)
        inv_p = persist.tile([128, P], f32)
        v = persist.tile([128, P], f32)
        dumpA = persist.tile([128, ACT_SPLIT], f32)   # ACT bisection dump
        dumpB = persist.tile([128, E], f32)           # DVE dump (norms/bisect/normsq)

        # ---------------- load qT (stationary operands) ----------------
        qpool = ctx.enter_context(tc.tile_pool(name="qtp", bufs=1))
        qts = []
        for k in range(KC):
            qt_k = qpool.tile([128, BC], f32, name=f"qt{k}", tag=f"qt{k}")
            nc.sync.dma_start(qt_k, qT[ts(k, 128), :])
            qts.append(qt_k)

        # PE warm-up: ~8us of back-to-back tiny matmuls so the HAM un-gates
        # the clock before the score matmuls start.  (Dead writes survive —
        # the BIR verifier only warns on no-reader locations.)
        with tc.tile_pool(name="pwarm", bufs=1, space="PSUM") as pw_pool:
            psw = pw_pool.tile([128, 128], f32)
            for i in range(32):
                nc.tensor.matmul(psw, qts[0], qts[1], start=True, stop=True)

        # Dummy 16-row gather issued up-front: forces Bacc's Q7 library load
        # (mlp, for DMAGatherAnt) to happen here instead of on the critical
        # path right before the real gather.
        idxw = persist.tile([128, (BC * S) // 16], mybir.dt.int16)
        nc.vector.memset(idxw[:, :1], 0)
        with tc.tile_pool(name="gwarm", bufs=1) as gw_pool:
            gwt = gw_pool.tile([128, 1, 64], f32)
            nc.gpsimd.dma_gather(out_ap=gwt, in_ap=pr0[:, :64],
                                 idxs_ap=idxw[:, :1], num_idxs=16,
                                 num_idxs_reg=16, elem_size=64, elem_step=LD)

        # ---------------- score = qT.T @ proj ----------------
        nsq_kb = persist.tile([128, PC], f32)
        sums8 = persist.tile([128, NSC], f32)
        with tc.tile_pool(name="projp", bufs=3) as proj_pool, \
             tc.tile_pool(name="pscore", bufs=1, space="PSUM") as ps_pool:
            ps = [ps_pool.tile([128, 512], f32, name=f"ps{n}", tag=f"ps{n}")
                  for n in range(NSC)]
            for k in range(KC):
                pj = proj_pool.tile([128, E], f32, name="pj", tag="pj")
                nc.sync.dma_start(pj, proj[ts(k, 128), :])
                for n in range(NSC):
                    nc.tensor.matmul(ps[n], qts[k], pj[:, ts(n, 512)],
                                     start=(k == 0), stop=(k == KC - 1))
            for n in range(NSC):
                # copy to SBUF and accumulate per-slice sums for the mean
                nc.vector.tensor_scalar(score[:, ts(n, 512)], ps[n], 1.0, None,
                                        op0=Alu.mult, op1=Alu.add,
                                        accum_out=sums8[:, n:n + 1])

        # ---------------- bisection for the 204th largest ----------------
        # Exact fp32 counting, split ACT (Sign with fused fp32 affine) + DVE.
        bis = ctx.enter_context(tc.tile_pool(name="bis", bufs=2))
        rs = bis.tile([128, 1], f32, name="rs", tag="rs", bufs=1)
        nc.vector.reduce_sum(rs, sums8, axis=mybir.AxisListType.X)
        lo = bis.tile([128, 1], f32, name="lo", tag="lo", bufs=1)
        nc.vector.tensor_scalar_mul(lo, rs, 1.0 / E)        # lo = rowmean
        hi = bis.tile([128, 1], f32, name="hi", tag="hi", bufs=1)
        nc.vector.reduce_max(hi, score, axis=mybir.AxisListType.X)

        DSPL = E - ACT_SPLIT
        # condition count>=204  <=>  cntD - 0.5*cntA >= 203.5 - ACT_SPLIT/2
        # where cntA = sum(sign(mid - x)) over the ACT half.
        thr = (float(NUM_ACTIVE) - 0.5) - ACT_SPLIT / 2.0
        with tc.tile_pool(name="pkeep", bufs=1, space="PSUM") as pk_pool:
            psk = pk_pool.tile([1, 128], f32)
            for _ in range(BISECT_ITERS):
                mid = bis.tile([128, 1], f32, name="mid", tag="mid")
                nc.vector.tensor_add(mid, lo, hi)
                nc.vector.tensor_scalar_mul(mid, mid, 0.5)
                cntA = bis.tile([128, 1], f32, name="cntA", tag="cntA")
                nc.scalar.activation(dumpA, score[:, :ACT_SPLIT], Act.Sign,
                                     bias=mid, scale=-1.0, accum_out=cntA)
                cntD = bis.tile([128, 1], f32, name="cntD", tag="cntD")
                nc.vector.tensor_scalar(dumpB[:, :DSPL], score[:, ACT_SPLIT:],
                                        mid, None, op0=Alu.is_ge, op1=Alu.add,
                                        accum_out=cntD)
                s_t = bis.tile([128, 1], f32, name="s_t", tag="s_t")
                nc.vector.scalar_tensor_tensor(s_t, cntA, -0.5, cntD,
                                               op0=Alu.mult, op1=Alu.add)
                g = bis.tile([128, 1], mybir.dt.uint8, name="g", tag="g")
                nc.vector.tensor_scalar(g, s_t, thr, None, op0=Alu.is_ge)
                gi = bis.tile([128, 1], mybir.dt.uint8, name="gi", tag="gi")
                nc.vector.tensor_scalar(gi, s_t, thr, None, op0=Alu.is_lt)
                nc.vector.copy_predicated(lo, g, mid)
                nc.vector.copy_predicated(hi, gi, mid)
                # keep the PE HAM warm through the bisection (reads mid so it
                # paces with the loop; result intentionally unused)
                nc.tensor.matmul(psk, mid, score[:, :128], start=True, stop=True)

        tpos = lo

        # key-bank loads on the ACT HWDGE ring (so they never head-of-line
        # block proj/kbT on the sync ring); issued right after the bisection,
        # consumed by DVE norm passes during the match phase.
        kbrow_pool = ctx.enter_context(tc.tile_pool(name="kbrows", bufs=3))
        kbrs = []
        for i in range(PC):
            kbr = kbrow_pool.tile([128, E], f32, name="kbr", tag="kbr")
            nc.scalar.dma_start(kbr, kb[ts(i, 128), :])
            kbrs.append(kbr)

        # ---------------- sparse (chunked so transposes can start early) ----
        for n in range(NSC):
            nc.vector.scalar_tensor_tensor(sparse[:, ts(n, 512)],
                                           score[:, ts(n, 512)], tpos,
                                           score[:, ts(n, 512)],
                                           op0=Alu.is_ge, op1=Alu.mult)

        # -------- interleaved: PE transpose of sparse + match matmul --------
        with tc.tile_pool(name="kbtp", bufs=6) as kbt_pool, \
             tc.tile_pool(name="ptr", bufs=3, space="PSUM") as ptr_pool, \
             tc.tile_pool(name="pm", bufs=1, space="PSUM") as pm_pool:
            psm = [pm_pool.tile([128, 512], f32, name=f"psm{n}", tag=f"psm{n}")
                   for n in range(2)]
            kts = {}
            for grp in range(EC // 4):
                pst4 = ptr_pool.tile([128, 512], f32, name="pst4", tag="pst4")
                for j in range(4):
                    e = grp * 4 + j
                    nc.tensor.transpose(pst4[:, ts(j, 128)],
                                        sparse[:, ts(e, 128)], ident)
                nc.vector.tensor_copy(
                    sparseT[:, grp * 4:(grp + 1) * 4, :].rearrange("p c i -> p (c i)"),
                    pst4)
                for j in range(4):
                    k = grp * 4 + j
                    kt = kbt_pool.tile([128, P], f32, name="kt", tag="kt")
                    nc.sync.dma_start(kt, kbT[ts(k, 128), :])
                    for n in range(2):
                        nc.tensor.matmul(psm[n], sparseT[:, k, :],
                                         kt[:, ts(n, 512)],
                                         start=(k == 0), stop=(k == EC - 1))

            # key-bank norms on DVE (fills DVE idle time under the match
            # matmul), then 1/max(||kb||,eps) broadcast to [128, P] via a
            # K=1 PE outer product.
            for i in range(PC):
                nc.vector.scalar_tensor_tensor(dumpB, kbrs[i], 1.0, kbrs[i],
                                               op0=Alu.bypass, op1=Alu.mult,
                                               accum_out=nsq_kb[:, i:i + 1])
            kn_norm = persist.tile([128, PC], f32)
            nc.scalar.activation(kn_norm, nsq_kb, Act.Sqrt)
            nc.vector.tensor_scalar_max(kn_norm, kn_norm, EPS)
            inv_kn = persist.tile([128, PC], f32)
            nc.vector.reciprocal(inv_kn, kn_norm)
            with tc.tile_pool(name="pinv", bufs=1, space="PSUM") as pinv_pool:
                inv_ps = pinv_pool.tile([PC, 128], f32)
                nc.tensor.transpose(inv_ps, inv_kn, ident)
                inv_knT = persist.tile([PC, 128], f32)
                nc.vector.tensor_copy(inv_knT, inv_ps)
                inv_row = persist.tile([1, P], f32)
                nc.sync.dma_start(inv_row.rearrange("a (c i) -> a c i", c=PC),
                                  inv_knT)
                ones_row = persist.tile([1, 128], f32)
                nc.vector.memset(ones_row, 1.0)
                for n in range(2):
                    inv_bc = pinv_pool.tile([128, 512], f32, name="inv_bc",
                                            tag="inv_bc")
                    nc.tensor.matmul(inv_bc, ones_row, inv_row[:, ts(n, 512)],
                                     start=True, stop=True)
                    nc.vector.tensor_copy(inv_p[:, ts(n, 512)], inv_bc)

            for n in range(2):
                nc.vector.tensor_tensor(v[:, ts(n, 512)], psm[n],
                                        inv_p[:, ts(n, 512)], op=Alu.mult)

        # ---------------- top-8 and gather indices ----------------
        max8 = persist.tile([128, S], f32)
        idx8 = persist.tile([128, S], mybir.dt.uint32)
        nc.vector.max(max8, v)
        nc.vector.max_index(idx8, max8, v)
        idx16 = persist.tile([128, S], mybir.dt.int16)
        nc.vector.tensor_copy(idx16, idx8)
        nc.sync.dma_start(idx_dram[:].rearrange("(b s) -> b s", s=S), idx16)
        # wrapped [16, 64] view, replicated to all 8 16-partition groups;
        # alternate the two HWDGE rings so the small loads run in parallel
        flat16 = idx_dram[:].rearrange("(s p) -> p s", p=16)
        for grp in range(8):
            eng = nc.sync if grp % 2 == 0 else nc.scalar
            eng.dma_start(idxw[ts(grp, 16), :], flat16)

        # ---------------- gather + 4-pool fan-out ----------------
        rows_per = (BC * S) // GATHER_CHUNKS                # 256
        slots = rows_per // 128                             # 2
        gpool = ctx.enter_context(tc.tile_pool(name="gath", bufs=2))
        for c in range(GATHER_CHUNKS):
            gt = gpool.tile([128, slots, LD], f32, name="gt", tag="gt")
            nc.gpsimd.dma_gather(
                out_ap=gt, in_ap=pr0[:],
                idxs_ap=idxw[:, c * (rows_per // 16):(c + 1) * (rows_per // 16)],
                num_idxs=rows_per, num_idxs_reg=rows_per, elem_size=LD)
            for pi in range(POOLS):
                dest = sel_out[pi, c * rows_per:(c + 1) * rows_per, :] \
                    .rearrange("(c2 p) e -> p c2 e", p=128)
                nc.sync.dma_start(dest, gt)

        # ---------------- similarity output (off the critical path) --------
        nsq_sp = bis.tile([128, 1], f32, name="nsq_sp", tag="nsq_sp", bufs=1)
        nc.vector.scalar_tensor_tensor(dumpB, sparse, 1.0, sparse,
                                       op0=Alu.bypass, op1=Alu.mult,
                                       accum_out=nsq_sp)
        sp_norm = bis.tile([128, 1], f32, name="sp_norm", tag="sp_norm", bufs=1)
        nc.scalar.activation(sp_norm, nsq_sp, Act.Sqrt)
        nc.vector.tensor_scalar_max(sp_norm, sp_norm, EPS)
        inv_sp = bis.tile([128, 1], f32, name="inv_sp", tag="inv_sp", bufs=1)
        nc.vector.reciprocal(inv_sp, sp_norm)
        simt = persist.tile([128, S], f32)
        nc.vector.tensor_scalar(simt, max8, inv_sp, None, op0=Alu.mult)
        nc.vector.tensor_scalar(simt, simt, -1.0, 1.0, op0=Alu.mult, op1=Alu.add)
        nc.sync.dma_start(sim_out[:], simt)


def _get_nc():
    if "nc" not in _CACHE:
        _CACHE["nc"] = _build_nc()
    return _CACHE["nc"]


def _run_device(q, proj, kb, pr0_flat):
    """One SPMD run over 8 cores. Returns (similarity [B,S], sel_rows [POOLS,B*S,LD] per-core list)."""
    from concourse.bass_utils import run_bass_kernel_spmd

    nc = _get_nc()
    kbT = np.ascontiguousarray(kb.T)
    in_maps = []
    for c in range(N_CORES):
        qs = q[c * BC:(c + 1) * BC]
        in_maps.append({
            "qT": np.ascontiguousarray(qs.T),
            "proj": proj,
            "kb": kb,
            "kbT": kbT,
            "pr0": pr0_flat,
        })
    res = run_bass_kernel_spmd(nc, in_maps, core_ids=list(range(N_CORES)))
    sims = np.concatenate([r["sim_out"] for r in res.results], axis=0)
    sel = np.concatenate(
        [r["sel_out"].reshape(POOLS, BC, S * L, D) for r in res.results], axis=1)
    return sims, sel


def kernel(query, random_projection, key_bank, prompts):
    q = np.ascontiguousarray(query, dtype=np.float32)
    proj = np.ascontiguousarray(random_projection, dtype=np.float32)
    kb = np.ascontiguousarray(key_bank, dtype=np.float32)
    prompts = np.asarray(prompts, dtype=np.float32)

    pools_same = all(np.array_equal(prompts[0], prompts[i]) for i in range(1, POOLS))
    pr0 = np.ascontiguousarray(prompts[0].reshape(P, LD))
    sims, sel = _run_device(q, proj, kb, pr0)
    if not pools_same:
        # Fallback (never hit with the reference setup_inputs): rerun per pool
        # and keep only that pool's slice from each run.
        for pi in range(1, POOLS):
            pri = np.ascontiguousarray(prompts[pi].reshape(P, LD))
            _, sel_i = _run_device(q, proj, kb, pri)
            sel[pi] = sel_i[pi]
    return sims, sel
